# revision 54
# baseline (speedup 1.0000x reference)
"""AttentionConv2d Trainium2 kernel — 8-core batch-data-parallel.

Each of the 8 NeuronCores processes one image of the batch:
  - qkv 1x1 conv + 3x3 conv as implicit-GEMM matmuls (bf16 compute, fp32 accum)
  - 8-head attention over 1024 positions with relative position biases folded
    into the K^T Q matmul via contract-dim augmentation (32 k-rows + 32 rel-H
    selector rows + 32 rel-W selector rows = 96-row contract)
  - rel-H / rel-W skews via 32 shifted-window matmuls per head per direction
    (the window slide through the reversed/plain rel kernel performs the skew
    during the matmul itself; no gather step needed)
  - softmax exp on ScalarE from PSUM; denominators ride the A*V matmul as 32
    appended ones-columns of V^T so they come out pre-broadcast
  - k-bias dropped (cancels in softmax), v-bias folded into the attn-conv
    bias host-side, q-bias applied on the qkv cast
  - 3x3 conv split into 8 quarter-chunks interleaved between attention heads
"""

import os
import sys

import numpy as np
import ml_dtypes

sys.path.insert(0, "/opt/trn_rl_repo")

B, C_IN, H, W = 8, 256, 32, 32
HW = H * W
DK = DV = 256
NH = 8
DKH = DK // NH  # 32
C_OUT = 512
N_CORES = 8

_CACHE = {}


def _build():
    import concourse.bass as bass
    import concourse.mybir as mybir
    import concourse.tile as tile
    from concourse import bacc
    from concourse.ap import AP
    from concourse.masks import make_identity
    from contextlib import ExitStack

    f32 = mybir.dt.float32
    bf16 = mybir.dt.bfloat16
    AF = mybir.ActivationFunctionType

    nc = bacc.Bacc("TRN2", target_bir_lowering=False, debug=False,
                   num_devices=N_CORES)

    x_d = nc.dram_tensor("x", [2, 128, HW], f32, kind="ExternalInput").ap()
    wqkv_d = nc.dram_tensor("wqkvT", [2, 128, 768], bf16, kind="ExternalInput").ap()
    wout_d = nc.dram_tensor("woutT", [2, 128, 9, 256], bf16, kind="ExternalInput").ap()
    wattn_d = nc.dram_tensor("wattnT", [2, 128, 256], bf16, kind="ExternalInput").ap()
    krw_d = nc.dram_tensor("krwT", [128, 63], bf16, kind="ExternalInput").ap()
    krh_d = nc.dram_tensor("krhTrev", [128, 63], bf16, kind="ExternalInput").ap()
    masks_d = nc.dram_tensor("masks", [64, 8, 128], bf16, kind="ExternalInput").ap()
    bias_d = nc.dram_tensor("bias", [128, 6], f32, kind="ExternalInput").ap()
    out_d = nc.dram_tensor("out", [4, 128, HW], f32, kind="ExternalOutput").ap()

    with tile.TileContext(nc) as tc, ExitStack() as ctx:
        wp = ctx.enter_context(tc.tile_pool(name="weights", bufs=1))
        ap_ = ctx.enter_context(tc.tile_pool(name="acts", bufs=1))
        hp = ctx.enter_context(tc.tile_pool(name="head", bufs=2))
        pbig = ctx.enter_context(tc.tile_pool(name="pbig", bufs=2, space="PSUM"))
        prelp = ctx.enter_context(tc.tile_pool(name="prelp", bufs=1, space="PSUM"))

        # ---- constants / weights to SBUF (qkv weights + x first) ----
        wqkv = wp.tile([128, 2, 768], bf16)
        wout = wp.tile([128, 2, 9, 256], bf16)
        wattn = wp.tile([128, 2, 256], bf16)
        krh = wp.tile([128, 63], bf16)   # reversed-row rel-H kernel, x4 stacked
        krw = wp.tile([128, 63], bf16)
        bias = wp.tile([128, 6], f32)    # q-bias x2 | attn-bias x2 | out-bias x2
        ident = wp.tile([128, 128], bf16)
        nc.sync.dma_start(wqkv[:], wqkv_d[:].rearrange("j p c -> p j c"))

        # ---- x: load f32, cast into zero-padded 34x34 bf16 image ----
        x32 = ap_.tile([128, 2, HW], f32)
        xp = ap_.tile([128, 2, 34 * 34], bf16)
        xpv = xp[:].rearrange("p j (y x) -> p j y x", y=34, x=34)
        for hf in range(2):
            for j in range(2):
                cols = slice(hf * 512, (hf + 1) * 512)
                nc.sync.dma_start(x32[:, j, cols], x_d[j][:, cols])
                nc.vector.tensor_copy(
                    xp[:, j, :].rearrange("p (y x) -> p y x", y=34, x=34)[
                        :, 1 + hf * 16:17 + hf * 16, 1:33],
                    x32[:, j, cols].rearrange("p (y x) -> p y x", y=16, x=32),
                )
        # borders only needed by the 3x3 conv chunks (first one at head 1)
        nc.vector.memset(xpv[:, :, 0:34:33, :], 0.0)   # top/bottom border rows
        nc.vector.memset(xpv[:, :, :, 0:34:33], 0.0)   # left/right border cols

        nc.sync.dma_start(krh[:], krh_d[:])
        nc.sync.dma_start(krw[:], krw_d[:])
        nc.sync.dma_start(bias[:], bias_d[:])
        nc.sync.dma_start(wout[:], wout_d[:].rearrange("j p t c -> p j t c"))
        nc.sync.dma_start(wattn[:], wattn_d[:].rearrange("j p c -> p j c"))
        make_identity(nc, ident[:])

        def xview(j, half, ky=1, kx=1, qy=0, ny=16):
            """[128, ny, 32] view of padded x, tap-shifted row window."""
            v = xp[:, j, :].rearrange("p (y x) -> p y x", y=34, x=34)
            y0 = half * 16 + qy + ky
            return v[:, y0: y0 + ny, kx: kx + 32]

        # ---- qkv = Wqkv @ x  (1x1 conv); ob 0,1=q 2,3=k 4,5=v ----
        qkv = ap_.tile([128, 6, 32, 32], bf16)

        def emit_qkv(ob):
            ps = pbig.tile([128, HW], f32, tag="big")
            for half in range(2):
                for j in range(2):
                    nc.tensor.matmul(
                        ps[:, half * 512:(half + 1) * 512],
                        wqkv[:, j, ob * 128:(ob + 1) * 128],
                        xview(j, half),
                        start=(j == 0), stop=(j == 1),
                    )
            flat = qkv[:, ob, :, :].rearrange("p y x -> p (y x)")
            if ob < 2:  # q: bias (k-bias cancels in softmax; v-bias folded)
                nc.vector.tensor_scalar_add(flat, ps[:], bias[:, ob:ob + 1])
            elif ob < 4:  # ScalarE is idle until the first exp
                nc.scalar.activation(flat, ps[:], AF.Copy)
            else:  # v: DVE so the transposes aren't gated on ScalarE
                nc.vector.tensor_copy(flat, ps[:])

        # tmpl lhsT: rows 0:32 k (per head), 32:64 U32, 64:96 I32
        tmplT = [ap_.tile([96, 8, 128], bf16, name=f"tmpl{i}") for i in range(2)]
        for t in tmplT:
            nc.sync.dma_start(t[32:96, :, :], masks_d[:])

        def emit_rel(h):
            """Rel-logit matmuls + rhs/tmpl assembly for head h."""
            hp0 = (h % 4) * 32
            j = h // 4
            qh = qkv[hp0:hp0 + 32, j, :, :]
            relp = prelp.tile([96, HW], f32, tag="relp")
            # rel-H skew via shifted windows of the reversed kernel:
            # relp[r, (y,x)] = q_(y,x) . KRH[62 - r - y] = G[y2 = 31 - r]
            for y in range(32):
                nc.tensor.matmul(
                    relp[0:32, y * 32:(y + 1) * 32],
                    krh[hp0:hp0 + 32, y: y + 32],
                    qh[:, y, :],
                    start=True, stop=True,
                    tile_position=(hp0, 0),
                )
            # rel-W (x-major): relp[64+x2, (x,y)] = q_(y,x) . KRW[x2 - x + 31]
            for x in range(32):
                nc.tensor.matmul(
                    relp[64:96, x * 32:(x + 1) * 32],
                    krw[hp0:hp0 + 32, 31 - x: 63 - x],
                    qh[:, :, x],
                    start=True, stop=True,
                    tile_position=(hp0, 64),
                )
            # rhs rows 0:32 q, 32:64 skewed rel-H, 64:96 rel-W
            rhs = hp.tile([96, HW], bf16, tag="rhs")
            nc.gpsimd.tensor_copy(rhs[0:32, :], qh.rearrange("p y x -> p (y x)"))
            nc.vector.tensor_copy(rhs[32:64, :], relp[0:32, :])
            nc.vector.tensor_copy(
                rhs[64:96, :].rearrange("p (y x) -> p y x", y=32, x=32),
                relp[64:96, :].rearrange("p (x y) -> p y x", x=32, y=32),
            )
            tm = tmplT[h % 2]
            kh = qkv[hp0:hp0 + 32, 2 + j, :, :]
            nc.gpsimd.tensor_copy(
                tm[0:32, :, :],
                kh.rearrange("p y x -> p (y x)").rearrange(
                    "p (m c) -> p m c", m=8, c=128),
            )
            return rhs, tm

        oconv = ap_.tile([128, 2, HW], f32)

        def emit_conv_chunk(c, act_cast=False):
            """One eighth of the 3x3 conv (256 output cols): 18 accum matmuls."""
            ob, half, q = c // 4, (c // 2) % 2, c % 2
            ps = pbig.tile([128, HW], f32, tag="big")
            for j in range(2):
                for t in range(9):
                    nc.tensor.matmul(
                        ps[:, 0:256],
                        wout[:, j, t, ob * 128:(ob + 1) * 128],
                        xview(j, half, t // 3, t % 3, qy=q * 8, ny=8),
                        start=((j, t) == (0, 0)), stop=((j, t) == (1, 8)),
                    )
            dst = oconv[:, ob, half * 512 + q * 256: half * 512 + (q + 1) * 256]
            if act_cast:
                nc.scalar.activation(dst, ps[:, 0:256], AF.Identity,
                                     bias=bias[:, 4 + ob:5 + ob])
            else:
                nc.vector.tensor_scalar_add(dst, ps[:, 0:256],
                                            bias[:, 4 + ob:5 + ob])
            if c % 4 == 3:
                nc.sync.dma_start(out_d[ob], oconv[:, ob, :])

        # prologue: q/k for head 0 first, then head-0 rel chain, rest of qkv
        emit_qkv(0)
        emit_qkv(2)
        pending = emit_rel(0)
        emit_qkv(4)
        emit_qkv(5)
        emit_qkv(1)
        emit_qkv(3)

        # ---- v^T with 32 ones-cols: vTe[., mb, h, 0:32]=v^T, [., 32:64]=1 ----
        # (transposes emitted inside head 0 as PE filler; pav opens after the
        # scoped transpose pool releases its 2 PSUM banks)
        vTe = ap_.tile([128, 8, 8, 64], bf16)
        nc.vector.memset(vTe[:, :, :, 32:64], 1.0)
        pavbox = {}

        def emit_vte():
            with tc.tile_pool(name="ptp", bufs=2, space="PSUM") as ptp:
                for mb in range(8):
                    for vb in range(2):
                        pt = ptp.tile([128, 128], bf16, tag="tp")
                        nc.tensor.transpose(
                            pt[:],
                            qkv[:, 4 + vb, :, :].rearrange("p y x -> p (y x)")[
                                :, mb * 128:(mb + 1) * 128],
                            ident[:],
                        )
                        nc.vector.tensor_copy(
                            vTe[:, mb, 4 * vb:4 * vb + 4, 0:32],
                            pt[:].rearrange("p (h d) -> p h d", h=4, d=32),
                        )
            pavbox["pav"] = ctx.enter_context(
                tc.tile_pool(name="pav", bufs=1, space="PSUM"))

        attn = ap_.tile([128, 2, HW], bf16)
        oattn0 = ap_.tile([128, 2, HW], f32)   # attn-conv j=0 partial
        sts, avps = {}, {}

        def qk(h, tm, rhs, mb):
            ps = pbig.tile([128, HW], f32, tag="big")
            for lh in range(2):
                nc.tensor.matmul(
                    ps[:, lh * 512:(lh + 1) * 512],
                    tm[:, mb, :],
                    rhs[:, lh * 512:(lh + 1) * 512],
                    start=True, stop=True,
                )
            nc.scalar.activation(sts[h][:, mb, :], ps[:], AF.Exp)

        def av(h, mb):
            for lh in range(2):
                nc.tensor.matmul(
                    avps[h][:, lh * 512:(lh + 1) * 512],
                    vTe[:, mb, h, :],
                    sts[h][:, mb, lh * 512:(lh + 1) * 512],
                    start=(mb == 0), stop=(mb == 7),
                )

        def norm(h):
            hq = (h % 4) * 32
            rec = hp.tile([32, HW], f32, tag="rec")
            nc.vector.reciprocal(rec[:], avps[h][32:64, :])
            nc.vector.tensor_mul(attn[hq:hq + 32, h // 4, :],
                                 avps[h][0:32, :], rec[:])

        # Software-pipelined head loop: head h-1's A*V tail and norm are
        # emitted between head h's first QK matmuls so the in-order PE queue
        # always has ready work while ScalarE drains the exp backlog.
        for h in range(NH):
            rhs, tm = pending
            sts[h] = hp.tile([128, 8, HW], bf16, tag="st", name=f"st{h}")
            if h > 0:
                av(h - 1, 5)
            qk(h, tm, rhs, 0)
            if h > 0:
                av(h - 1, 6)
            qk(h, tm, rhs, 1)
            if h > 0:
                av(h - 1, 7)
            if h < 7:  # rel first: its DVE copies gate next head's PE rel mms
                pending = emit_rel(h + 1)
            if h > 0:
                norm(h - 1)
                emit_conv_chunk(h - 1)
            if h == 0:
                emit_vte()
            qk(h, tm, rhs, 2)
            qk(h, tm, rhs, 3)
            avps[h] = pavbox["pav"].tile([64, HW], f32, tag="av",
                                         name=f"avp{h}")
            av(h, 0)
            if h in (4, 5):  # heads 0-3 done: j=0 half of the attn 1x1 conv
                ob = h - 4
                ps = pbig.tile([128, HW], f32, tag="big")
                for lh in range(2):
                    nc.tensor.matmul(
                        ps[:, lh * 512:(lh + 1) * 512],
                        wattn[:, 0, ob * 128:(ob + 1) * 128],
                        attn[:, 0, lh * 512:(lh + 1) * 512],
                        start=True, stop=True,
                    )
                nc.scalar.activation(oattn0[:, ob, :], ps[:], AF.Copy)
            qk(h, tm, rhs, 4)
            av(h, 1)
            qk(h, tm, rhs, 5)
            av(h, 2)
            qk(h, tm, rhs, 6)
            av(h, 3)
            qk(h, tm, rhs, 7)
            av(h, 4)

        av(7, 5)
        av(7, 6)
        av(7, 7)
        emit_conv_chunk(7, act_cast=True)  # fills PE while DVE starts norm(7)

        # ---- tail: lh-split norm(7) hand-interleaved with the attn 1x1
        # conv j=1 half + merge, so the DVE chain never bubbles ----
        oattn = ap_.tile([128, 2, HW], f32)
        rec7 = hp.tile([32, HW], f32, tag="rec")
        pss = [pbig.tile([128, HW], f32, tag="big", name=f"psat{ob}")
               for ob in range(2)]
        l0, l1 = slice(0, 512), slice(512, 1024)

        def t_recip(lh):
            c = (l0, l1)[lh]
            nc.vector.reciprocal(rec7[:, c], avps[7][32:64, c])

        def t_mult(lh):
            c = (l0, l1)[lh]
            nc.vector.tensor_mul(attn[96:128, 1, c], avps[7][0:32, c],
                                 rec7[:, c])

        def t_mm(ob, lh):
            c = (l0, l1)[lh]
            nc.tensor.matmul(pss[ob][:, c], wattn[:, 1, ob * 128:(ob + 1) * 128],
                             attn[:, 1, c], start=True, stop=True)

        def t_stt(ob, lh):
            c = (l0, l1)[lh]
            nc.vector.scalar_tensor_tensor(
                oattn[:, ob, c], pss[ob][:, c], bias[:, 2 + ob:3 + ob],
                oattn0[:, ob, c], mybir.AluOpType.add, mybir.AluOpType.add)
            nc.sync.dma_start(out_d[2 + ob][:, c], oattn[:, ob, c])

        t_recip(0)
        t_mult(0)
        t_mm(0, 0)
        t_mm(1, 0)
        t_recip(1)
        t_stt(0, 0)
        t_mult(1)
        t_mm(0, 1)
        t_mm(1, 1)
        t_stt(1, 0)
        t_stt(0, 1)
        t_stt(1, 1)

    nc.compile()
    return nc


def _host_inputs(x, w_qkv, b_qkv, w_attn, b_attn, w_out, b_out,
                 key_rel_w, key_rel_h):
    bf = ml_dtypes.bfloat16
    s = DKH ** -0.5
    wq = np.asarray(w_qkv, np.float32)[:, :, 0, 0].copy()   # [768, 256]
    bqf = np.asarray(b_qkv, np.float32).copy()
    wq[:DK] *= s
    bqf[:DK] *= s
    wqkvT = np.ascontiguousarray(wq.T).reshape(2, 128, 768).astype(bf)
    wattn2 = np.asarray(w_attn, np.float32)[:, :, 0, 0]
    wattnT = np.ascontiguousarray(wattn2.T).reshape(2, 128, 256).astype(bf)
    woutT = np.ascontiguousarray(
        np.asarray(w_out, np.float32).transpose(1, 2, 3, 0).reshape(256, 9, 256)
    ).reshape(2, 128, 9, 256).astype(bf)
    krwT = np.ascontiguousarray(
        np.tile(np.asarray(key_rel_w, np.float32).T, (4, 1))).astype(bf)
    krhTrev = np.ascontiguousarray(
        np.tile(np.asarray(key_rel_h, np.float32)[::-1, :].T, (4, 1))).astype(bf)

    masks = np.zeros((64, 8, 128), np.float32)
    for mb in range(8):
        for jj in range(128):
            y2 = mb * 4 + jj // 32
            masks[31 - y2, mb, jj] = 1.0       # U32 (diag DMA stores y2=31-r)
    for jj in range(128):
        masks[32 + jj % 32, :, jj] = 1.0       # I32
    masks = masks.astype(bf)

    bqm = bqf[:DK].reshape(2, 128).T                                  # [128, 2]
    battn_eff = (np.asarray(b_attn, np.float32)
                 + wattn2 @ np.asarray(b_qkv, np.float32)[2 * DK:])
    battn = battn_eff.reshape(2, 128).T
    boutm = np.asarray(b_out, np.float32).reshape(2, 128).T
    bias = np.ascontiguousarray(
        np.concatenate([bqm, battn, boutm], axis=1))                  # [128, 6]

    shared = dict(wqkvT=wqkvT, wattnT=wattnT, woutT=woutT, krwT=krwT,
                  krhTrev=krhTrev, masks=masks, bias=bias)
    xs = np.asarray(x, np.float32).reshape(B, 2, 128, HW)
    return [dict(shared, x=np.ascontiguousarray(xs[i])) for i in range(N_CORES)]


def kernel(**inputs):
    from concourse.bass_utils import run_bass_kernel_spmd
    if "nc" not in _CACHE:
        _CACHE["nc"] = _build()
    nc = _CACHE["nc"]
    in_maps = _host_inputs(**inputs)
    res = run_bass_kernel_spmd(nc, in_maps, list(range(N_CORES)),
                               trace=bool(os.environ.get("BASS_KERNEL_TRACE")))
    _CACHE["last_result"] = res
    outs = [r["out"].reshape(C_OUT, H, W) for r in res.results]
    return np.stack(outs).astype(np.float32)
